# revision 24
# baseline (speedup 1.0000x reference)
"""Trainium2 Bass kernel for nn_PixelEachSubstitutor (8-core data parallel).

v2 design (all fp32; bf16 rejected — the net amplifies rounding ~30x):
  - encP: 3 column-streams (7 b2-blocks each), emission-interleaved.
  - A->B remap: PE transpose of zp -> DRAM zq[(b2,t)][(bt,c)] with 484B
    contiguous runs -> per-(stream,btL) reads (vs 4B-granule scatter DMAs).
  - encL: 4 streams over b2-blocks; j enumerated (b2-major, bt-minor) so
    the DMA runs stay contiguous; pad seqs (bt=10, b2>=15) dropped via
    110-element short-block runs.  Layer 0 runs from the 9 real canvas
    tokens (18 partitions).
  - B->C remap: per-stream PE transposes -> zr[(stream j, c)][(btL,h)]
    -> per-btC interval reads, overlapped with other streams' compute.
  - encC: btC = 12 x 19 jG-intervals; chunked attention pipeline.
  - Score chain: mul/sv split across DVE/GpSimd (rates ~1.04 vs ~2.43
    ns/elem), exp on ACT, segmented reduces on DVE.
"""
import os
import sys

for _p in ("/opt/trn_rl_repo", os.path.expanduser("~/.axon_site/_ro/trn_rl_repo")):
    if os.path.isdir(_p) and _p not in sys.path:
        sys.path.insert(0, _p)

import numpy as np

NUM_CLASSES = 10
D_PAD = 11
L = 49
EPS = 1e-5
BC = 225
P_BT, P_B2 = 11, 21      # P: 121 partitions, free (b2, t)
F_P = P_B2 * 9           # 189
REAL9 = [0, 1, 2, 7, 8, 9, 14, 15, 16]
C_BT, C_B2 = 12, 19      # C: 120 partitions, free (jC, h)
F_C = C_B2 * L           # 931

# P streams: column ranges; stream si feeds transpose chunk si
P_STREAMS = [(0, 7), (7, 14), (14, 21)]
# which L-stream zl9 reads to emit in each p_stream tail
P_TAIL_READS = [[0], [1], [2, 3]]

# L streams: b2-block ranges
STREAM_B2 = [(0, 6), (6, 11), (11, 16), (16, 21)]


def block_nbt(b2):
    return 10 if b2 >= 15 else 11


def stream_tables():
    tabs = []
    for lo, hi in STREAM_B2:
        seqs = [(b2, bt) for b2 in range(lo, hi) for bt in range(block_nbt(b2))]
        j0 = (len(seqs) + 1) // 2
        tabs.append((seqs, j0))
    return tabs


TABS = stream_tables()
STREAM_J = [len(t[0]) for t in TABS]            # [66, 55, 54, 50]
STREAM_J0 = [t[1] for t in TABS]                # btL0 j counts [33, 28, 27, 25]
JG_BASE = np.cumsum([0] + STREAM_J).tolist()    # [0, 66, 121, 175, 225]
ZR_BASE = np.cumsum([0] + STREAM_J0).tolist()   # zr row-block base per stream


def jg_locate(jg):
    """jG -> (stream, btL, j_local)"""
    for s in range(4):
        if jg < JG_BASE[s + 1]:
            jl = jg - JG_BASE[s]
            j0 = STREAM_J0[s]
            return (s, 0, jl) if jl < j0 else (s, 1, jl - j0)
    raise ValueError(jg)


def stream_dma_segments(s, g):
    """Read-DMA segments for (L-stream s, btL g): list of
    (j_slot_off, b2_start, nb2, bt_lo, bt_hi) with uniform block shape;
    j_slot_off is the dst offset in 11-slot units within this btL row-group."""
    seqs, j0 = TABS[s]
    jlist = seqs[:j0] if g == 0 else seqs[j0:]
    segs = []
    i = 0
    while i < len(jlist):
        b2, bt = jlist[i]
        # run of consecutive bt within this b2
        k = i
        while k + 1 < len(jlist) and jlist[k + 1] == (jlist[k][0], jlist[k][1] + 1):
            k += 1
        bt_hi = jlist[k][1] + 1
        segs.append([i, b2, 1, bt, bt_hi])
        i = k + 1
    # merge consecutive full blocks (same bt range, consecutive b2)
    merged = [segs[0]]
    for seg in segs[1:]:
        m = merged[-1]
        if (seg[3], seg[4]) == (m[3], m[4]) and seg[1] == m[1] + m[2] \
                and seg[4] - seg[3] == block_nbt(m[1]) == block_nbt(seg[1]):
            m[2] += 1
        else:
            merged.append(seg)
    return [tuple(x) for x in merged]


def zc_dma_segments(btc):
    """zc read segments for btC: list of (jc_off, nj, zr_row_base, half)"""
    glo = btc * C_B2
    ghi = min(glo + C_B2, BC)
    segs = []
    g = glo
    while g < ghi:
        s, btl, jl = jg_locate(g)
        lim = JG_BASE[s] + (STREAM_J0[s] if btl == 0 else STREAM_J[s])
        n = min(ghi, lim) - g
        segs.append((g - glo, n, (ZR_BASE[s] + jl) * 10, btl))
        g += n
    return segs


# --------------------------------------------------------------------------
# host-side staging
# --------------------------------------------------------------------------

def build_x0(x_full, core):
    N, C, H, W = x_full.shape
    xp = np.zeros((N, C + 1, H + 2, W + 2), np.float32)
    xp[:, :C, 1:H + 1, 1:W + 1] = x_full
    xp[:, C, :, :] = 1.0
    xp[:, C, 1:H + 1, 1:W + 1] = 0.0
    x0 = np.zeros((121, F_P), np.float32)
    for bl in range(BC):
        bg = BC * core + bl
        n, i, j = np.unravel_index(bg, (N, H, W))
        bt, b2 = bl // P_B2, bl % P_B2
        for t in range(9):
            di, dj = t // 3, t % 3
            x0[bt * 11:bt * 11 + 11, b2 * 9 + t] = xp[n, :, i + di, j + dj]
    return x0


def assemble_output(y_cores):
    out = np.zeros((2, NUM_CLASSES, 30, 30), np.float32)
    for core, y in enumerate(y_cores):
        for jg in range(BC):
            s, btl, jl = jg_locate(jg)
            b2, bt = TABS[s][0][jl + (0 if btl == 0 else STREAM_J0[s])]
            bl = bt * 21 + b2
            bg = BC * core + bl
            n, i, j = np.unravel_index(bg, (2, 30, 30))
            btc, jc = jg // C_B2, jg % C_B2
            out[n, :, i, j] = y[btc * 10:btc * 10 + 10, jc]
    return out


# --------------------------------------------------------------------------
# weight packing
# --------------------------------------------------------------------------

def center(d):
    return np.eye(d, dtype=np.float32) - np.full((d, d), 1.0 / d, np.float32)


def bd(A, n):
    return np.kron(np.eye(n, dtype=np.float32), A.astype(np.float32))


class Pack:
    def __init__(self):
        self.off = {}
        self.n = 0
        self.mats = []

    def add(self, name, mat):
        K, M = mat.shape
        assert K <= 128 and M <= 128, (name, mat.shape)
        self.off[name] = (self.n, K, M)
        self.mats.append(mat.astype(np.float32))
        self.n += M

    def array(self):
        a = np.zeros((128, self.n), np.float32)
        for (c0, K, M), m in zip(self.off.values(), self.mats):
            a[:K, c0:c0 + M] = m
        return a


def build_packs(W):
    pk = Pack()
    # ---- encP ----
    C11 = center(D_PAD)
    for l in range(6):
        Wq, Wk, Wv = W['pWin'][l][:11], W['pWin'][l][11:22], W['pWin'][l][22:]
        pk.add(f"Pq{l}", bd(Wq.T, P_BT))
        pk.add(f"Pk{l}", bd(Wk.T, P_BT))
        pk.add(f"Pv{l}", bd(Wv.T, P_BT))
        pk.add(f"Pwo{l}", bd((C11 @ W['pWout'][l]).T, P_BT))
        for m, grp in enumerate(([0, 1, 2, 3], [4, 5, 6, 7], [8, 9, 10])):
            f1 = np.zeros((121, 32 * len(grp)), np.float32)
            f2 = np.zeros((32 * len(grp), 121), np.float32)
            cf2 = C11 @ W['pWf2'][l]
            for gi, bt in enumerate(grp):
                f1[bt * 11:bt * 11 + 11, gi * 32:(gi + 1) * 32] = W['pWf1'][l].T
                f2[gi * 32:(gi + 1) * 32, bt * 11:bt * 11 + 11] = cf2.T
            pk.add(f"Pf1{l}_{m}", f1)
            pk.add(f"Pf2{l}_{m}", f2)
    pk.add("PC", bd(C11, P_BT))
    pk.add("Pones", bd(np.ones((11, 1), np.float32), P_BT))
    pk.add("Pbc", bd(np.ones((1, 11), np.float32), P_BT))
    sel_s = np.zeros((121, 11), np.float32)
    sel_p = np.zeros((121, 11), np.float32)
    for bt in range(P_BT):
        sel_s[bt * 11:bt * 11 + 10, bt] = 1.0
        sel_p[bt * 11 + 10, bt] = 1.0
    pk.add("PselS", sel_s)
    pk.add("PselP", sel_p)
    # ---- encL ----
    C49 = center(L)
    for l in range(6):
        Wq, Wk, Wv = W['LWin'][l][:49], W['LWin'][l][49:98], W['LWin'][l][98:]
        pk.add(f"Lq{l}", bd(Wq.T, 2))
        pk.add(f"Lk{l}", bd(Wk.T, 2))
        pk.add(f"Lv{l}", bd(Wv.T, 2))
        pk.add(f"Lwo{l}", bd((C49 @ W['LWout'][l]).T, 2))
        pk.add(f"Lf1{l}", bd(W['LWf1'][l].T, 2))
        pk.add(f"Lf2{l}", bd((C49 @ W['LWf2'][l]).T, 2))
    pk.add("LC", bd(C49, 2))
    pk.add("Lones", bd(np.ones((49, 1), np.float32), 2))
    pk.add("Lbc", bd(np.ones((1, 49), np.float32), 2))
    # 9-token layer-0 variants: rows (btL, t9)
    Wq, Wk, Wv = W['LWin'][0][:49], W['LWin'][0][49:98], W['LWin'][0][98:]
    for nm, Wx in (("Lq9", Wq), ("Lk9", Wk), ("Lv9", Wv)):
        m = np.zeros((18, 98), np.float32)
        for g in range(2):
            m[g * 9:g * 9 + 9, g * 49:g * 49 + 49] = Wx.T[REAL9, :]
        pk.add(nm, m)
    m = np.zeros((18, 98), np.float32)
    for g in range(2):
        m[g * 9:g * 9 + 9, g * 49:g * 49 + 49] = C49[REAL9, :]
    pk.add("LC9", m)
    # ---- encC ----
    C10 = center(NUM_CLASSES)
    Wq, Wk, Wv = W['CWin'][0][:10], W['CWin'][0][10:20], W['CWin'][0][20:]
    pk.add("Cq0", bd(Wq.T, C_BT))
    pk.add("Ck0", bd(Wk.T, C_BT))
    pk.add("Cv0", bd(Wv.T, C_BT))
    pk.add("Cwo0", bd((C10 @ W['CWout'][0]).T, C_BT))
    pk.add("Cf10", bd(W['CWf1'][0].T, C_BT))
    pk.add("Cf20", bd((C10 @ W['CWf2'][0]).T, C_BT))
    pk.add("CC", bd(C10, C_BT))
    pk.add("Cones", bd(np.ones((10, 1), np.float32), C_BT))
    pk.add("Cbc", bd(np.ones((1, 10), np.float32), C_BT))
    pk.add("wdrep", np.tile(W['Wdec'][0][None, :], (120, 1)))
    pk.add("eye", np.eye(128, dtype=np.float32))
    # ---- vecs ----
    NV = 27
    vecs = np.zeros((128, NV), np.float32)
    vecs[:, 26] = EPS
    for l in range(6):
        vecs[:121, 2 * l] = np.tile(W['pln1'][l], P_BT)
        vecs[:121, 2 * l + 1] = np.tile(W['pln2'][l], P_BT)
        vecs[:98, 12 + 2 * l] = np.tile(W['Lln1'][l], 2)
        vecs[:98, 12 + 2 * l + 1] = np.tile(W['Lln2'][l], 2)
    vecs[:120, 24] = np.tile(W['Cln1'][0], C_BT)
    vecs[:120, 25] = np.tile(W['Cln2'][0], C_BT)
    return pk, vecs


# --------------------------------------------------------------------------
# device kernel
# --------------------------------------------------------------------------

def _patch_tail_drain(tile_mod, ScopedClock, VectorClock):
    if getattr(tile_mod.TileContext, "_tail_patched", False):
        return

    def _drain_and_barrier(self, tick_clock, wait_clock):
        gc = tick_clock.global_clock
        n = len(gc)
        for i in range(n):
            t = gc[i]
            if t <= 0:
                continue
            vec = [0] * n
            vec[i] = t
            d = self.nc.sync.drain()
            wait_clock.add_sem_waits(d.ins, ScopedClock({None: VectorClock(vec)}))
        self.nc.sync.drain()
        self.nc.all_engine_barrier()
        assert self.sems is not None
        popped = self.nc._tile_sem_poison_stack.pop()
        assert popped is self._sem_poison
        self.nc.clear_and_free_semaphores(list(self.sems.allocated().values()))
        self.nc.all_engine_barrier()

    tile_mod.TileContext._drain_and_barrier = _drain_and_barrier
    tile_mod.TileContext._tail_patched = True


def interleave(gens, width=None, stagger=0):
    gens = list(gens)
    if width is None:
        width = len(gens)
    alive = []
    nxt = 0
    turn = 0
    while alive or nxt < len(gens):
        while len(alive) < width and nxt < len(gens) \
                and turn >= nxt * stagger:
            alive.append(gens[nxt])
            nxt += 1
        if not alive:
            turn += 1
            continue
        done = []
        for g in alive:
            try:
                next(g)
            except StopIteration:
                done.append(g)
        for g in done:
            alive.remove(g)
        turn += 1


def build_bass_program():
    import concourse.bass as bass
    import concourse.mybir as mybir
    import concourse.tile as tile_mod
    import concourse.tile_sem_assignment as tsa
    from concourse.vector_clock import ScopedClock, VectorClock

    _patch_tail_drain(tile_mod, ScopedClock, VectorClock)
    tsa.NUM_HWDGE_SEMS = 1

    f32 = mybir.dt.float32
    f32r = mybir.dt.float32r
    AF = mybir.ActivationFunctionType
    ALU = mybir.AluOpType
    AX = mybir.AxisListType

    pk, vecs_arr = _PACKS
    NW = pk.n

    nc = bass.Bass("TRN2", target_bir_lowering=False, debug=False, num_devices=1)
    x0_d = nc.dram_tensor("x0", [121, F_P], f32, kind="ExternalInput")
    wp_d = nc.dram_tensor("wp", [128, NW], f32r, kind="ExternalInput")
    vec_d = nc.dram_tensor("vecs", [128, 27], f32, kind="ExternalInput")
    y_d = nc.dram_tensor("y", [120, C_B2], f32, kind="ExternalOutput")
    zq_d = nc.dram_tensor("zq_scr", [189 * 121], f32, kind="Internal")
    zr_d = nc.dram_tensor("zr_scr", [113 * 10 * 98], f32, kind="Internal")

    def APX(t, free_dims, extra_off=0):
        pstep, pcnt = t.ap[0]
        return bass.AP(tensor=t.tensor, offset=t.offset + extra_off,
                       ap=[[pstep, pcnt]] + free_dims)

    with tile_mod.TileContext(nc) as tc:

        def wap(name):
            c0, K, M = pk.off[name]
            return wtile[0:K, c0:c0 + M]

        def eyeap(n):
            c0, K, M = pk.off["eye"]
            return wtile[0:n, c0:c0 + n]

        C0W = pk.off["Cq0"][0]
        NCW = NW - C0W

        with tc.tile_pool(name="persist", bufs=1) as persist:
            vtile = persist.tile([128, 27], f32)
            nc.sync.dma_start(vtile[:], vec_d[:])
            wtileC = persist.tile([128, NCW], f32)
            nc.sync.dma_start(wtileC[:].bitcast(f32r),
                              bass.AP(tensor=wp_d, offset=C0W,
                                      ap=[[NW, 128], [1, NCW]]))

            def wapC(name):
                c0, K, M = pk.off[name]
                return wtileC[0:K, c0 - C0W:c0 - C0W + M]

            # ---------------- generic building blocks ----------------
            def mm(ps_ap, lhsT, rhs, start, stop, r):
                if r:
                    lhsT = lhsT.bitcast(f32r)
                    rhs = rhs.bitcast(f32r)
                nc.tensor.matmul(ps_ap, lhsT, rhs, start=start, stop=stop)

            def FR(ap, r):
                return ap.bitcast(f32r) if r else ap

            def attention(pools, x_ap, pre, l, parts, ntok, nj, o_out, r,
                          eng_mul, eng_sv, sfx="", x9=False, qkv_bufs=1,
                          rx=None, inplace_sv=True):
                """One attention chunk: nj sequences of ntok tokens."""
                ps, ss, sbig = pools['ps'], pools['ss'], pools['sbig']
                bigtag = pools['bigtag']
                cn = nj * ntok
                g = cn * ntok
                wfn = pools.get('wap', wap)
                qn = f"{pre}q9" if x9 else f"{pre}q{l}"
                kn = f"{pre}k9" if x9 else f"{pre}k{l}"
                if rx is None:
                    rx = r
                qkv_bufs = pools.get('bigbufs', qkv_bufs)
                qt = ps.tile([parts, cn], f32, tag=bigtag, name="qt",
                             bufs=qkv_bufs)
                kt = ps.tile([parts, cn], f32, tag=bigtag, name="kt",
                             bufs=qkv_bufs)
                vt = ps.tile([parts, cn], f32, tag=bigtag, name="vt",
                             bufs=qkv_bufs)
                qps, kps, vps = qt[:, :], kt[:, :], vt[:, :]
                vn = f"{pre}v9" if x9 else f"{pre}v{l}"
                mm(qps, wfn(qn), x_ap, True, True, rx)
                yield
                mm(kps, wfn(kn), x_ap, True, True, rx)
                yield
                mm(vps, wfn(vn), x_ap, True, True, rx)
                yield
                ksb = ss.tile([parts, cn], f32, tag="ks" + sfx, name="ksb",
                              bufs=2)
                nc.scalar.copy(ksb[:, :], kps)
                yield
                vsb = ss.tile([parts, cn], f32, tag="vs" + sfx, name="vsb",
                              bufs=2)
                nc.scalar.copy(vsb[:, :], vps)
                yield
                s = sbig.tile([parts, g], f32, tag="s" + sfx, name="s")
                s4 = APX(s, [[ntok * ntok, nj], [ntok, ntok], [1, ntok]])
                k4 = APX(ksb, [[ntok, nj], [0, ntok], [1, ntok]])
                if eng_mul == 'g':
                    qsb = ss.tile([parts, cn], f32, tag="qs" + sfx, name="qsb")
                    nc.scalar.copy(qsb[:, :], qps)
                    yield
                    q4 = APX(qsb, [[ntok, nj], [1, ntok], [0, ntok]])
                    nc.gpsimd.tensor_mul(s4, k4, q4)
                else:
                    q4 = bass.AP(tensor=qt.tensor, offset=qt.offset,
                                 ap=[list(qt.ap[0]),
                                     [ntok, nj], [1, ntok], [0, ntok]])
                    nc.vector.tensor_mul(s4, k4, q4)
                yield
                nc.scalar.activation(s[:, :], s[:, :], AF.Exp)
                yield
                s3 = APX(s, [[ntok, cn], [1, ntok]])
                den = ss.tile([parts, cn], f32, tag="den" + sfx, name="den",
                              bufs=2)
                nc.vector.tensor_reduce(den[:, :], s3, axis=AX.X, op=ALU.add)
                yield
                if inplace_sv:
                    svt, sv4, sv3 = s, s4, s3
                else:
                    svt = sbig.tile([parts, g], f32, tag="sv" + sfx,
                                    name="svt")
                    sv4 = APX(svt, [[ntok * ntok, nj], [ntok, ntok],
                                    [1, ntok]])
                    sv3 = APX(svt, [[ntok, cn], [1, ntok]])
                v4 = APX(vsb, [[ntok, nj], [0, ntok], [1, ntok]])
                if eng_sv == 'g':
                    nc.gpsimd.tensor_mul(sv4, s4, v4)
                else:
                    nc.vector.tensor_mul(sv4, s4, v4)
                yield
                num = ss.tile([parts, cn], f32, tag="num" + sfx, name="num",
                              bufs=2)
                nc.vector.tensor_reduce(num[:, :], sv3, axis=AX.X, op=ALU.add)
                yield
                nc.scalar.activation(den[:, :], den[:, :], AF.Ln)
                yield
                nc.scalar.activation(den[:, :], den[:, :], AF.Exp, scale=-1.0)
                yield
                if eng_sv == 'g':
                    nc.gpsimd.tensor_mul(FR(o_out, r), num[:, :], den[:, :])
                else:
                    nc.vector.tensor_mul(FR(o_out, r), num[:, :], den[:, :])
                yield

            def layer_norm(pools, terms, ffn, parts, bt, d, F, chunks,
                           w_ap, Cn, ones, bc, out_sb, r, sfx="", r0=None):
                ps, ss = pools['ps'], pools['ss']
                lntag, bigtag = pools['lntag'], pools['bigtag']
                xc = ss.tile([parts, F], f32, tag="xc" + sfx, name="xc")
                sq = ss.tile([parts, F], f32, tag="sq" + sfx, name="sq")
                sd = ss.tile([bt, F], f32, tag="sd" + sfx, name="sd")
                n = len(terms)
                for (c0, cn) in chunks:
                    xps = ps.tile([parts, cn], f32, tag=lntag, name="xps")
                    if r0 is None:
                        r0 = r
                    wfn = pools.get('wap', wap)
                    for i, (lhsT, rhs_fn) in enumerate(terms):
                        lh = Cn if lhsT is None else wfn(lhsT)
                        mm(xps[:, :], lh, rhs_fn(c0, cn),
                           i == 0, (i == n - 1) and not ffn,
                           r0 if i == 0 else r)
                        yield
                    x_in_fn = terms[0][1]
                    for j, (f1name, f2name, hparts) in enumerate(ffn):
                        hps = ps.tile([hparts, cn], f32, tag=bigtag,
                                      name="hps", bufs=pools.get('bigbufs', 1))
                        mm(hps[:, :], wfn(f1name), x_in_fn(c0, cn), True,
                           True, r)
                        yield
                        hsb = ss.tile([hparts, cn], f32, tag="hs" + sfx,
                                      name="hsb")
                        nc.scalar.activation(FR(hsb[:, :], r), hps[:, :],
                                             AF.Relu)
                        yield
                        mm(xps[:, :], wfn(f2name), hsb[:, :],
                           False, j == len(ffn) - 1, r)
                        yield
                    nc.scalar.copy(xc[:, c0:c0 + cn], xps[:, :])
                    yield
                    nc.scalar.activation(FR(sq[:, c0:c0 + cn], r), xps[:, :],
                                         AF.Square)
                    yield
                    vps = ps.tile([bt, cn], f32, tag=lntag, name="vps")
                    mm(vps[:, :], ones, sq[:, c0:c0 + cn], True, True, r)
                    yield
                    nc.scalar.activation(FR(sd[:, c0:c0 + cn], r), vps[:, :],
                                         AF.Ln, bias=vtile[0:bt, 26:27],
                                         scale=1.0 / d)
                    yield
                nc.scalar.activation(FR(sd[:, :], r), sd[:, :], AF.Exp,
                                     scale=-0.5)
                yield
                for (c0, cn) in chunks:
                    bps = ps.tile([parts, cn], f32, tag=lntag, name="bps")
                    mm(bps[:, :], bc, sd[:, c0:c0 + cn], True, True, r)
                    yield
                    for _ in range(pools.get('stt_delay', 0)):
                        yield
                    stt_e = nc.gpsimd if pools.get('stt_g') else nc.vector
                    stt_e.scalar_tensor_tensor(
                        out=FR(out_sb[:, c0:c0 + cn], r),
                        in0=xc[:, c0:c0 + cn],
                        scalar=w_ap, in1=bps[:, :],
                        op0=ALU.mult, op1=ALU.mult)
                    yield

            # ================= stage A: encP (2 streams) =================
            with tc.tile_pool(name="sbZ", bufs=1) as sbZ:
                zl9s = []
                for s in range(4):
                    J0 = STREAM_J0[s]
                    zl9 = sbZ.tile([18, J0 * 11], f32, tag=f"zl9{s}",
                                   name="zl9")
                    nc.gpsimd.memset(zl9[:, :], 0.0)
                    zl9s.append(zl9)
                zc = sbZ.tile([120, 932], f32, tag="zc", name="zc")
                nc.gpsimd.memset(zc[:, :], 0.0)

                def emit_zl9_reads(ls):
                    for g in range(2):
                        for (joff, b2s, nb2, btlo, bthi) in \
                                stream_dma_segments(ls, g):
                            run = (bthi - btlo) * 11
                            src = bass.AP(
                                tensor=zq_d,
                                offset=(b2s * 9) * 121 + btlo * 11,
                                ap=[[121, 9], [1089, nb2], [1, run]])
                            dst = APX(
                                zl9s[ls][g * 9:g * 9 + 9, :],
                                [[block_nbt(b2s) * 11, nb2], [1, run]],
                                extra_off=joff * 11)
                            nc.sync.dma_start(dst, src)

                zc_segs_by_stream = {s: [] for s in range(4)}
                for btc_ in range(C_BT):
                    for seg_ in zc_dma_segments(btc_):
                        s_ = jg_locate(btc_ * C_B2 + seg_[0])[0]
                        zc_segs_by_stream[s_].append((btc_, seg_))

                def emit_zc_reads(s):
                    for btc, (jcoff, nj, rowbase, half) in \
                            zc_segs_by_stream[s]:
                        src = bass.AP(tensor=zr_d,
                                      offset=rowbase * 98 + half * 49,
                                      ap=[[98, 10], [980, nj], [1, 49]])
                        dst = APX(zc[btc * 10:btc * 10 + 10, :],
                                  [[49, nj], [1, 49]], extra_off=jcoff * 49)
                        nc.sync.dma_start(dst, src)

                wpool_cm = tc.tile_pool(name="wpool", bufs=1)
                wpool = wpool_cm.__enter__()
                wtile = wpool.tile([128, NW], f32, name="wtile")
                nc.sync.dma_start(wtile[:].bitcast(f32r), wp_d[:])

                with tc.tile_pool(name="sbP", bufs=1) as sbP, \
                     tc.tile_pool(name="ssP", bufs=2) as ssP, \
                     tc.tile_pool(name="sgP", bufs=1) as sgP, \
                     tc.tile_pool(name="psP", bufs=1, space="PSUM") as psP:
                    x0t = sbP.tile([121, F_P], f32, tag="x0", name="x0t")
                    nc.sync.dma_start(x0t[:], x0_d[:])
                    zp = sbP.tile([121, F_P], f32, tag="zp", name="zp")

                    def hidP(l):
                        return [(f"Pf1{l}_0", f"Pf2{l}_0", 128),
                                (f"Pf1{l}_1", f"Pf2{l}_1", 128),
                                (f"Pf1{l}_2", f"Pf2{l}_2", 96)]

                    def p_stream(si):
                        blo, bhi = P_STREAMS[si]
                        c0s, cns = blo * 9, (bhi - blo) * 9
                        nj = bhi - blo
                        sfx = f"P{si}"
                        pools = {'ps': psP, 'ss': ssP, 'sbig': sgP,
                                 'bigtag': "bigP", 'lntag': f"ln{sfx}",
                                 'bigbufs': 4}
                        x = x0t
                        xoff = c0s
                        for l in range(6):
                            o = ssP.tile([121, cns], f32, tag="o" + sfx,
                                         name="o")
                            yield from attention(
                                pools, x[:, xoff:xoff + cns], "P", l, 121, 9,
                                nj, o[:, :], False, 'v', 'g', sfx=sfx)
                            x1 = ssP.tile([121, cns], f32, tag="x1" + sfx,
                                          name="x1")
                            yield from layer_norm(
                                pools,
                                [(None, lambda c0, cn, xx=x, xo=xoff:
                                  xx[:, xo + c0:xo + c0 + cn]),
                                 (f"Pwo{l}", lambda c0, cn, oo=o:
                                  oo[:, c0:c0 + cn])],
                                [], 121, P_BT, D_PAD, cns, [(0, cns)],
                                vtile[0:121, 2 * l:2 * l + 1], wap("PC"),
                                wap("Pones"), wap("Pbc"), x1, False, sfx=sfx)
                            x2 = ssP.tile([121, cns], f32, tag="x2" + sfx,
                                          name="x2")
                            yield from layer_norm(
                                pools,
                                [(None, lambda c0, cn, xx=x1:
                                  xx[:, c0:c0 + cn])],
                                hidP(l), 121, P_BT, D_PAD, cns, [(0, cns)],
                                vtile[0:121, 2 * l + 1:2 * l + 2], wap("PC"),
                                wap("Pones"), wap("Pbc"), x2, False, sfx=sfx)
                            x = x2
                            xoff = 0
                        # ---- z build ----
                        eh = ssP.tile([121, cns], f32, tag="eh" + sfx,
                                      name="eh")
                        nc.scalar.activation(eh[:, :], x[:, :], AF.Exp)
                        yield
                        dps = psP.tile([11, cns], f32, tag=f"ln{sfx}",
                                       name="dps")
                        nc.tensor.matmul(dps[:, :], wap("PselS"), eh[:, :])
                        yield
                        mps = psP.tile([11, cns], f32, tag=f"ln{sfx}",
                                       name="mps")
                        nc.tensor.matmul(mps[:, :], wap("PselP"),
                                         x0t[:, c0s:c0s + cns])
                        yield
                        denr = ssP.tile([11, cns], f32, tag="denr" + sfx,
                                        name="denr")
                        nc.scalar.activation(denr[:, :], dps[:, :], AF.Ln)
                        yield
                        nc.scalar.activation(denr[:, :], denr[:, :], AF.Exp,
                                             scale=-1.0)
                        yield
                        scl = ssP.tile([11, cns], f32, tag="scl" + sfx,
                                       name="scl")
                        nc.vector.tensor_mul(scl[:, :], denr[:, :], mps[:, :])
                        yield
                        sps = psP.tile([121, cns], f32, tag=f"ln{sfx}",
                                       name="sps")
                        nc.tensor.matmul(sps[:, :], wap("Pbc"), scl[:, :])
                        yield
                        nc.vector.tensor_mul(zp[:, c0s:c0s + cns], eh[:, :],
                                             sps[:, :])
                        yield
                        nc.gpsimd.tensor_add(zp[:, c0s:c0s + cns],
                                             zp[:, c0s:c0s + cns],
                                             x0t[:, c0s:c0s + cns])
                        yield
                        # ---- A->B for this column chunk + L reads ----
                        tp = psP.tile([cns, 121], f32, tag=f"ln{sfx}",
                                      name="tp")
                        nc.tensor.matmul(tp[:, :], zp[:, c0s:c0s + cns],
                                         eyeap(121), is_transpose=True)
                        yield
                        tsb = sbP.tile([cns, 121], f32, tag=f"tsb{si}",
                                       name="tsb")
                        nc.scalar.copy(tsb[:, :], tp[:, :])
                        yield
                        dst = bass.AP(tensor=zq_d, offset=c0s * 121,
                                      ap=[[121, cns], [1, 121]])
                        nc.sync.dma_start(dst, tsb[:, :])
                        yield
                        for ls in P_TAIL_READS[si]:
                            emit_zl9_reads(ls)
                            yield

                    interleave([p_stream(si) for si in range(3)])

                # ================= stage B: encL (4 streams) =============
                with tc.tile_pool(name="ssL", bufs=1) as ssL, \
                     tc.tile_pool(name="sgL", bufs=1) as sgL, \
                     tc.tile_pool(name="psL", bufs=1, space="PSUM") as psL:

                    def l_stream(s):
                        J0 = STREAM_J0[s]
                        F10 = J0 * 10
                        sfx = f"L{s}"
                        pools = {'ps': psL, 'ss': ssL, 'sbig': sgL,
                                 'bigtag': "qkvL", 'lntag': f"ln{sfx}",
                                 'bigbufs': 4}
                        x = None
                        for l in range(6):
                            o = ssL.tile([98, F10], f32, tag="o" + sfx,
                                         name="o", bufs=2)
                            if l == 0:
                                x_ap = APX(zl9s[s], [[11, J0], [1, 10]])
                            else:
                                x_ap = x[:, :]
                            eng_m = 'v' if (s + l) % 2 == 0 else 'g'
                            eng_s = 'g' if (s + l) % 2 == 0 else 'v'
                            yield from attention(
                                pools, x_ap, "L", l, 98, 10, J0,
                                o[:, :], True, eng_m, eng_s,
                                sfx=sfx, x9=(l == 0), rx=(l > 0),
                                qkv_bufs=3)
                            x1 = ssL.tile([98, F10], f32, tag="x1" + sfx,
                                          name="x1", bufs=2)
                            if l == 0:
                                t0 = [("LC9", lambda c0, cn, ss_=s:
                                       APX(zl9s[ss_],
                                           [[11, cn // 10], [1, 10]],
                                           extra_off=c0 // 10 * 11))]
                            else:
                                t0 = [(None, lambda c0, cn, xx=x:
                                       xx[:, c0:c0 + cn])]
                            yield from layer_norm(
                                pools,
                                t0 + [(f"Lwo{l}", lambda c0, cn, oo=o:
                                       oo[:, c0:c0 + cn])],
                                [], 98, 2, L, F10, [(0, F10)],
                                vtile[0:98, 12 + 2 * l:13 + 2 * l],
                                wap("LC"), wap("Lones"), wap("Lbc"), x1,
                                True, sfx=sfx, r0=(l > 0))
                            x2 = ssL.tile([98, F10], f32, tag="x2" + sfx,
                                          name="x2", bufs=2)
                            yield from layer_norm(
                                pools,
                                [(None, lambda c0, cn, xx=x1:
                                  xx[:, c0:c0 + cn])],
                                [(f"Lf1{l}", f"Lf2{l}", 2)], 98, 2, L, F10,
                                [(0, F10)],
                                vtile[0:98, 13 + 2 * l:14 + 2 * l], wap("LC"),
                                wap("Lones"), wap("Lbc"), x2, True, sfx=sfx)
                            x = x2
                        # ---- B->C transpose + zr write + zc reads ----
                        nch = (F10 + 119) // 120
                        for ci in range(nch):
                            c0 = ci * 120
                            cw = min(120, F10 - c0)
                            tp = psL.tile([cw, 98], f32, tag=f"ln{sfx}",
                                          name="tpL")
                            nc.tensor.matmul(tp[:, :], x[:, c0:c0 + cw],
                                             eyeap(98), is_transpose=True)
                            yield
                            tsb = ssL.tile([cw, 98], f32, tag="tsb" + sfx,
                                           name="tsbL")
                            nc.scalar.copy(tsb[:, :], tp[:, :])
                            yield
                            dst = bass.AP(tensor=zr_d,
                                          offset=(ZR_BASE[s] * 10 + c0) * 98,
                                          ap=[[98, cw], [1, 98]])
                            nc.sync.dma_start(dst, tsb[:, :])
                            yield
                        emit_zc_reads(s)
                        yield

                    interleave([l_stream(s) for s in range(4)])

                wpool_cm.__exit__(None, None, None)

                # ================= stage C =================
                with tc.tile_pool(name="sbC", bufs=1) as sbC, \
                     tc.tile_pool(name="ssC", bufs=2) as ssC, \
                     tc.tile_pool(name="sgC", bufs=1) as sgC, \
                     tc.tile_pool(name="psC", bufs=1, space="PSUM") as psC:
                    CCHUNKS = [(0, 2), (2, 2), (4, 2), (6, 2), (8, 2),
                               (10, 2), (12, 2), (14, 2), (16, 2), (18, 1)]
                    oC = sbC.tile([120, 932], f32, tag="oC", name="oC")

                    def c_chunk(ci):
                        j0, nj = CCHUNKS[ci]
                        eng_m = 'g' if ci % 2 == 0 else 'v'
                        eng_s = 'g' if ci % 2 == 1 else 'v'
                        pools = {'ps': psC, 'ss': ssC, 'sbig': sgC,
                                 'bigtag': "qkvC", 'lntag': "lnC",
                                 'bigbufs': 3, 'wap': wapC}
                        yield from attention(
                            pools, zc[:, j0 * 49:(j0 + nj) * 49], "C", 0,
                            120, L, nj, oC[:, j0 * 49:(j0 + nj) * 49], True,
                            eng_m, eng_s, sfx=f"C{ci % 2}", qkv_bufs=3,
                            rx=False, inplace_sv=False)

                    interleave([c_chunk(ci) for ci in range(len(CCHUNKS))],
                               width=2)

                    def c_tail():
                        pools = {'ps': psC, 'ss': ssC, 'sbig': sgC,
                                 'bigtag': "hidC", 'lntag': "lnC",
                                 'wap': wapC}
                        x1 = sbC.tile([120, 932], f32, tag="x1C", name="x1C")
                        yield from layer_norm(
                            pools,
                            [(None, lambda c0, cn: zc[:, c0:c0 + cn]),
                             ("Cwo0", lambda c0, cn: oC[:, c0:c0 + cn])],
                            [], 120, C_BT, NUM_CLASSES, 932,
                            [(0, 466), (466, 466)],
                            vtile[0:120, 24:25], wapC("CC"), wapC("Cones"),
                            wapC("Cbc"), x1, True, sfx="Ct", r0=False)
                        x2 = sbC.tile([120, 932], f32, tag="x2C", name="x2C")
                        yield from layer_norm(
                            pools,
                            [(None, lambda c0, cn: x1[:, c0:c0 + cn])],
                            [("Cf10", "Cf20", 12)], 120, C_BT, NUM_CLASSES,
                            932, [(0, 466), (466, 466)],
                            vtile[0:120, 25:26], wapC("CC"), wapC("Cones"),
                            wapC("Cbc"), x2, True, sfx="Ct")
                        # decode
                        wd = wapC("wdrep")
                        tprod = sbC.tile([120, F_C], f32, tag="tp",
                                         name="tprod")
                        wd3 = APX(wd, [[0, C_B2], [1, 49]])
                        x3 = APX(x2, [[49, C_B2], [1, 49]])
                        t3 = APX(tprod, [[49, C_B2], [1, 49]])
                        nc.gpsimd.tensor_mul(t3, x3, wd3)
                        yield
                        ytile = sbC.tile([120, C_B2], f32, tag="y",
                                         name="ytile")
                        nc.vector.tensor_reduce(
                            ytile[:, :], APX(tprod, [[49, C_B2], [1, 49]]),
                            axis=AX.X, op=ALU.add)
                        yield
                        nc.sync.dma_start(y_d[:], ytile[:, :])
                        yield

                    interleave([c_tail()])

    import bass_rust as _bass_rust
    _bass_rust.move_matmul_waits_to_ldweights(nc.m)
    _bass_rust.generate_event_semaphores(nc)
    return nc


_PACKS = None


def _install_ntff_hook():
    import types
    try:
        import antenv.axon_hooks  # noqa: F401
        return
    except ImportError:
        pass
    try:
        from trn_agent_boot.trn_boot import _ntff_profile_via_ctypes
    except ImportError:
        sys.path.insert(0, os.path.expanduser("~/.axon_site"))
        from trn_agent_boot.trn_boot import _ntff_profile_via_ctypes
    hook = None
    for so in ("/opt/axon/libaxon_pjrt.so",):
        if os.path.exists(so):
            hook = _ntff_profile_via_ctypes(so)
            break
    mod = types.ModuleType("antenv.axon_hooks")
    mod.get_axon_ntff_profile_hook = lambda: hook
    mod.set_axon_ntff_profile_hook = lambda h: None
    import antenv
    antenv.axon_hooks = mod
    sys.modules["antenv.axon_hooks"] = mod


def kernel(**inputs):
    global _PACKS
    W = {k: np.asarray(v, np.float32) for k, v in inputs.items()}
    x_full = W.pop('x')
    pk, vecs_arr = build_packs(W)
    _PACKS = (pk, vecs_arr)
    wpack_arr = pk.array()

    nc = build_bass_program()

    from concourse.bass_utils import run_bass_kernel_spmd
    trace = os.environ.get("KERNEL_TRACE", "") == "1"
    if trace:
        _install_ntff_hook()
    in_maps = []
    for core in range(8):
        in_maps.append({
            "x0": build_x0(x_full, core),
            "wp": wpack_arr,
            "vecs": vecs_arr,
        })
    res = run_bass_kernel_spmd(nc, in_maps, core_ids=list(range(8)),
                               trace=trace)
    kernel.last_result = res
    ys = [res.results[i]["y"] for i in range(8)]
    return assemble_output(ys)


if __name__ == "__main__":
    rng = np.random.default_rng(0)
    print("building program only (syntax check)...")
    W = {
        'pWin': rng.standard_normal((6, 33, 11)), 'pWout': rng.standard_normal((6, 11, 11)),
        'pWf1': rng.standard_normal((6, 32, 11)), 'pWf2': rng.standard_normal((6, 11, 32)),
        'pln1': np.ones((6, 11)), 'pln2': np.ones((6, 11)),
        'LWin': rng.standard_normal((6, 147, 49)), 'LWout': rng.standard_normal((6, 49, 49)),
        'LWf1': rng.standard_normal((6, 1, 49)), 'LWf2': rng.standard_normal((6, 49, 1)),
        'Lln1': np.ones((6, 49)), 'Lln2': np.ones((6, 49)),
        'CWin': rng.standard_normal((1, 30, 10)), 'CWout': rng.standard_normal((1, 10, 10)),
        'CWf1': rng.standard_normal((1, 1, 10)), 'CWf2': rng.standard_normal((1, 10, 1)),
        'Cln1': np.ones((1, 10)), 'Cln2': np.ones((1, 10)),
        'Wdec': rng.standard_normal((1, 49)),
    }
    W = {k: np.asarray(v, np.float32) for k, v in W.items()}
    pk, vecs_arr = build_packs(W)
    _PACKS = (pk, vecs_arr)
    print("wpack cols:", pk.n)
    nc = build_bass_program()
    print("program built OK")
